# revision 1
# baseline (speedup 1.0000x reference)
"""CAML-style multi-label attention kernel for Trainium2 (8 NeuronCores).

Reference computation (B=8, W=1000, V=50000, E=100, C=50, K=3, L=18000):
    emb    = W_embed[x]                            (B, W, E)
    H      = tanh(conv1d(emb, conv_w) + conv_b)    (B, W, C)  'same' padding
    scores = einsum("lc,bwc->blw", u_w, H)
    attns  = softmax(scores, axis=w)
    m      = einsum("blw,bwc->blc", attns, H)
    out    = sigmoid(sum(out_w * m, axis=c) + out_b)   (B, L)

Sharding: L=18000 split across 8 cores (2250 labels each, padded to 2304).
Embedding + conv prologue is replicated on every core (it is tiny).

Per-core device algorithm (all f32, f32r fast matmuls):
  - gather embedding rows with indirect DMA (tokens -> partitions)
  - PE-transpose to (E, W) layout, conv as 3 accumulating matmuls, tanh
  - keep H in both (C, W) layout (Hcw) and transposed augmented layout
    Haug[(w, [h_0..h_49, 1])] per 128-token chunk; the ones column makes the
    pooling matmul also produce the softmax denominator.
  - scores computed TRANSPOSED: scT[w, l] = sum_c Hcw[c, w] * u_wT[c, l]
    (no transpose needed: both operands already have c on partitions)
  - exp on ScalarE (no max subtraction: |scores| is bounded ~O(5) here)
  - pooling: m_aug[c', l] = sum_w Haug[w, c'] * exp_scT[w, l]; row 50 = denom
  - transpose m_aug back to label-partition layout (tiny), then the final
    out_w dot, divide, + bias, sigmoid(z) = 1/(1+exp(-z)) on Vector/Scalar
    (sigmoid ACT table-set would thrash against exp's table-set).
"""

import numpy as np

try:
    import concourse.bass as bass
except ImportError:  # repo not on sys.path in fresh dirs
    import sys

    sys.path.insert(0, "/opt/trn_rl_repo")
    import concourse.bass as bass

import concourse.bacc as bacc
import concourse.tile as tile
from concourse import mybir
from concourse.bass import IndirectOffsetOnAxis
from concourse.bass_utils import run_bass_kernel_spmd
from concourse.masks import make_identity

FP = mybir.dt.float32
FR = mybir.dt.float32r
BF = mybir.dt.bfloat16
AF = mybir.ActivationFunctionType

B, W, V, E, C, K, L = 8, 1000, 50000, 100, 50, 3, 18000
NCORES = 8
WPAD = 1024  # W padded to 8 chunks of 128
LSH = L // NCORES  # 2250 labels per core
LPAD = 2304  # 18 tiles of 128
LT = LPAD // 128  # 18 label tiles per core
NCI = WPAD // 128  # 8 w-chunks


import os

EMB_HOST = bool(int(os.environ.get("EMB_HOST", "0")))
LBLOCKS = int(os.environ.get("LBLOCKS", "5"))
PREC_CONV = os.environ.get("PREC_CONV", "f32r")
PREC_MM1 = os.environ.get("PREC_MM1", "f32r")
PREC_MM2 = os.environ.get("PREC_MM2", "f32r")


def _p(ap, prec):
    """Pick matmul operand dtype: f32r (fast, tf32-ish) or fp32 (exact, 4x)."""
    return ap.bitcast(FR if prec == "f32r" else FP)


def build_nc(num_devices: int, repeat: int = 1):
    nc = bacc.Bacc(
        "TRN2", target_bir_lowering=False, debug=False, num_devices=num_devices
    )

    x_idx = nc.dram_tensor("x_idx", [128, B * NCI], mybir.dt.int32, kind="ExternalInput").ap()
    embx = (
        nc.dram_tensor("embx", [B, 128, NCI * E], FP, kind="ExternalInput").ap()
        if EMB_HOST
        else None
    )
    wemb = nc.dram_tensor("wemb", [V, E], FP, kind="ExternalInput").ap()
    convwt = nc.dram_tensor("convwt", [E, K * C], BF, kind="ExternalInput").ap()
    convb = nc.dram_tensor("convb", [C, 1], FP, kind="ExternalInput").ap()
    uwt = nc.dram_tensor("uwt", [128, LPAD], BF, kind="ExternalInput").ap()
    owp = nc.dram_tensor("owp", [128, LT * C], FP, kind="ExternalInput").ap()
    obp = nc.dram_tensor("obp", [128, LT], FP, kind="ExternalInput").ap()
    out = nc.dram_tensor("out", [B, 128, LT], FP, kind="ExternalOutput").ap()

    with tile.TileContext(nc) as tc:
        for _ in range(repeat):
            _body(tc, nc, x_idx, wemb, convwt, convb, uwt, owp, obp, out, embx=embx)
    nc.compile()
    return nc


def _body(tc, nc, x_idx, wemb, convwt, convb, uwt, owp, obp, out, dbg=None, embx=None):
    with (
        tc.tile_pool(name="const", bufs=1) as constp,
        tc.tile_pool(name="work", bufs=2) as workp,
        tc.tile_pool(name="expp", bufs=3) as expp,
        tc.tile_pool(name="psA", bufs=2, space="PSUM") as psA,  # sc: 2 banks x2
        tc.tile_pool(name="psB", bufs=1, space="PSUM") as psB,  # ma: 1 bank
        tc.tile_pool(name="psT", bufs=2, space="PSUM") as psT,  # ptm: 1 bank x2
        tc.tile_pool(name="psP", bufs=1, space="PSUM") as psP,  # prologue: 1 bank
    ):
        ident = constp.tile([128, 128], FP, name="ident")
        make_identity(nc, ident)
        ident_bf = constp.tile([128, 128], BF, name="ident_bf")
        make_identity(nc, ident_bf)

        uwt_s = constp.tile([128, LPAD], BF, name="uwt_s")
        nc.sync.dma_start(out=uwt_s, in_=uwt)
        convwt_s = constp.tile([E, K * C], BF, name="convwt_s")
        nc.sync.dma_start(out=convwt_s, in_=convwt)
        convb_s = constp.tile([C, 1], FP, name="convb_s")
        nc.sync.dma_start(out=convb_s, in_=convb)
        owp_s = constp.tile([128, LT * C], FP, name="owp_s")
        nc.sync.dma_start(out=owp_s, in_=owp)
        obp_s = constp.tile([128, LT], FP, name="obp_s")
        nc.sync.dma_start(out=obp_s, in_=obp)
        idx_s = constp.tile([128, B * NCI], mybir.dt.int32, name="idx_s")
        nc.sync.dma_start(out=idx_s, in_=x_idx)

        # persistent per-batch H in both layouts
        Hcw = constp.tile([128, B * WPAD], BF, name="Hcw")
        Haug = constp.tile([128, B * NCI * (C + 1)], BF, name="Haug")

        # memset is not encodable for f32r tiles: stage constants in fp32 and
        # copy (copy with f32r out dtype is the sanctioned "rounded" producer)
        zeros_c = constp.tile([128, 32], FP, name="zeros_c")
        nc.gpsimd.memset(zeros_c, 0.0)
        ones_c = constp.tile([128, 1], FP, name="ones_c")
        nc.gpsimd.memset(ones_c, 1.0)

        # ---- per batch: prologue (gather/conv/layouts) then label blocks ---
        for b in range(B):
            # embP[e, 1+w] = emb[w, e]; col 0 and cols 1001.. are zero padding
            embP = workp.tile([E, 1032], BF, tag="embP", name="embP")
            nc.gpsimd.memset(embP[:, 0:1], 0.0)
            nc.gpsimd.memset(embP[:, 1001:1032], 0.0)
            if embx is not None:
                embh = workp.tile([128, NCI * E], FP, tag="embh", name="embh")
                nc.sync.dma_start(out=embh, in_=embx[b])
            for ci in range(NCI):
                if embx is None:
                    # HW DGE only handles one index per partition reliably:
                    # gather 128 embedding rows per call
                    emb_g = workp.tile([128, E], BF, tag="embg", name="emb_g", bufs=12)
                    nc.gpsimd.indirect_dma_start(
                        out=emb_g[:, :],
                        out_offset=None,
                        in_=wemb[:, :],
                        in_offset=IndirectOffsetOnAxis(
                            ap=idx_s[:, b * NCI + ci : b * NCI + ci + 1], axis=0
                        ),
                    )
                else:
                    emb_g = embh[:, ci * E : (ci + 1) * E]
                pt = psP.tile([128, 128], BF, tag="pp", name="pt")
                nc.tensor.transpose(
                    out=pt[:E, :], in_=emb_g[:, :], identity=ident_bf[:, :]
                )
                cw = min(128, W - ci * 128)
                nc.vector.tensor_copy(
                    out=embP[:, 1 + ci * 128 : 1 + ci * 128 + cw], in_=pt[:E, :cw]
                )

            # conv1d: H[c, w] = tanh(sum_k convw_k.T @ embP[:, w+k] + b)
            for w0, cw in ((0, 512), (512, W - 512)):
                pm = psP.tile([C + 1, 512], FP, tag="pp", name="convps")
                for k in range(K):
                    nc.tensor.matmul(
                        out=pm[:C, :cw],
                        lhsT=convwt_s[:, k * C : (k + 1) * C],
                        rhs=embP[:, w0 + k : w0 + k + cw],
                        start=(k == 0),
                        stop=(k == K - 1),
                    )
                nc.scalar.activation(
                    out=Hcw[:C, b * WPAD + w0 : b * WPAD + w0 + cw],
                    in_=pm[:C, :cw],
                    func=AF.Tanh,
                    bias=convb_s[:, 0:1],
                )
            nc.gpsimd.memset(Hcw[:C, b * WPAD + W : (b + 1) * WPAD], 0.0)
            # duplicate H rows at partitions 64..113 for row-packed mm1
            nc.sync.dma_start(
                out=Hcw[64 : 64 + C, b * WPAD : (b + 1) * WPAD],
                in_=Hcw[:C, b * WPAD : (b + 1) * WPAD],
            )

            # Haug[w, 0:50] = H[w, :]; col 50 = 1.0 (0.0 on pad rows)
            for ci in range(NCI):
                pt2 = psP.tile([128, 256], BF, tag="pp", name="pt2")
                nc.tensor.transpose(
                    out=pt2[:, :C],
                    in_=Hcw[:C, b * WPAD + ci * 128 : b * WPAD + (ci + 1) * 128],
                    identity=ident_bf[:C, :C],
                )
                base = (b * NCI + ci) * (C + 1)
                nc.vector.tensor_copy(out=Haug[:, base : base + C], in_=pt2[:, :C])
                if ci < NCI - 1:
                    nc.gpsimd.memset(Haug[:, base + C : base + C + 1], 1.0)
                else:
                    # last chunk: 1.0 only on the W-896 valid token rows
                    nc.gpsimd.memset(Haug[:, base + C : base + C + 1], 0.0)
                    nc.gpsimd.memset(Haug[: W - 896, base + C : base + C + 1], 1.0)

            # ------------- main: label blocks for this batch ----------------
            # per-batch label-partition results: [p, lt*64 + (0..49 m, 50 s)]
            mlt = workp.tile([128, LT * 64], FP, tag="mlt", name="mlt")

            def lb_post(ma_u, lb_u, LW_u):
                # back to label-partition layout via small PE transposes,
                # packed 4-per-bank so one strided copy moves them all
                msb = workp.tile([C + 1, 512], FP, tag="msb", name="msb")
                nc.vector.tensor_copy(out=msb[:, :LW_u], in_=ma_u[:, :LW_u])
                nq = LW_u // 128
                ptm = psT.tile([128, 256], FP, tag="pt", name="ptm")
                for q in range(nq):
                    nc.tensor.transpose(
                        out=ptm[:, q * 64 : q * 64 + C + 1],
                        in_=msb[:, q * 128 : (q + 1) * 128],
                        identity=ident[: C + 1, : C + 1],
                    )
                nc.vector.tensor_copy(
                    out=mlt.rearrange("p (t s) -> p t s", s=64)[
                        :, lb_u * 4 : lb_u * 4 + nq, 0 : C + 1
                    ],
                    in_=ptm.rearrange("p (q s) -> p q s", s=64)[:, 0:nq, 0 : C + 1],
                )

            def emit_mm2(u):
                ma_u, ex_u, lb_u, LW_u, pair_u = u
                for h in range(2):
                    ci = pair_u * 2 + h
                    base = (b * NCI + ci) * (C + 1)
                    nc.tensor.matmul(
                        out=ma_u[:, :LW_u],
                        lhsT=Haug[:, base : base + C + 1],
                        rhs=ex_u[:, h * 512 : h * 512 + LW_u],
                        start=(ci == 0),
                        stop=(ci == NCI - 1),
                    )
                if pair_u == 3:
                    lb_post(ma_u, lb_u, LW_u)

            # one-stage software pipeline: emit mm2 of the previous pair after
            # this pair's mm1+exp, so PE is never queued behind a stalled mm2
            pend = None
            for lb in range(LBLOCKS):
                lb0 = lb * 512
                LW = min(512, LPAD - lb0)
                ma = psB.tile([C + 1, 512], FP, tag="maug", name="ma")
                for pair in range(4):
                    sc = psA.tile([128, 1024], FP, tag="sc", name="sc")
                    ex = expp.tile([128, 1024], BF, tag="ex", name="ex")
                    for h in range(2):
                        # two concurrent 50-row contractions in distinct PE
                        # row groups (partitions 0-49 and 64-113)
                        ci = pair * 2 + h
                        p0 = 64 * h
                        nc.tensor.matmul(
                            out=sc[:, h * 512 : h * 512 + LW],
                            lhsT=Hcw[
                                p0 : p0 + C,
                                b * WPAD + ci * 128 : b * WPAD + (ci + 1) * 128,
                            ],
                            rhs=uwt_s[p0 : p0 + C, lb0 : lb0 + LW],
                            start=True,
                            stop=True,
                        )
                    if LW == 512:
                        nc.scalar.activation(out=ex[:, :], in_=sc[:, :], func=AF.Exp)
                    else:
                        for h in range(2):
                            nc.scalar.activation(
                                out=ex[:, h * 512 : h * 512 + LW],
                                in_=sc[:, h * 512 : h * 512 + LW],
                                func=AF.Exp,
                            )
                    if pend is not None:
                        emit_mm2(pend)
                    pend = (ma, ex, lb, LW, pair)
            if pend is not None:
                emit_mm2(pend)
                pend = None

            # final: d = sum_c m*out_w; sigmoid((d/s) + bias)
            m3 = mlt.rearrange("p (t q) -> p t q", q=64)
            prod = workp.tile([128, LT * C], FP, tag="prod", name="prod")
            nc.vector.tensor_mul(
                out=prod.rearrange("p (t c) -> p t c", c=C),
                in0=m3[:, :, 0:C],
                in1=owp_s.rearrange("p (t c) -> p t c", c=C),
            )
            d = workp.tile([128, LT], FP, tag="d", name="d")
            nc.vector.tensor_reduce(
                out=d,
                in_=prod.rearrange("p (t c) -> p t c", c=C),
                axis=mybir.AxisListType.X,
                op=mybir.AluOpType.add,
            )
            rs = workp.tile([128, LT], FP, tag="rs", name="rs")
            nc.vector.reciprocal(out=rs, in_=m3[:, :, C : C + 1])
            dz = workp.tile([128, LT], FP, tag="dz", name="dz")
            nc.vector.tensor_mul(out=dz, in0=d, in1=rs)
            zt = workp.tile([128, LT], FP, tag="zt", name="zt")
            nc.vector.tensor_add(out=zt, in0=dz, in1=obp_s)
            ez = workp.tile([128, LT], FP, tag="ez", name="ez")
            nc.scalar.activation(out=ez, in_=zt, func=AF.Exp, scale=-1.0)
            e1 = workp.tile([128, LT], FP, tag="e1", name="e1")
            nc.vector.tensor_scalar_add(e1, ez, 1.0)
            osb = workp.tile([128, LT], FP, tag="osb", name="osb")
            nc.vector.reciprocal(out=osb, in_=e1)
            nc.sync.dma_start(out=out[b, :, :], in_=osb)
            if dbg is not None:
                nc.sync.dma_start(
                    out=dbg[2][b, :, :],
                    in_=mlt.rearrange("p (t q) -> p t q", q=64)[:, :, 0 : C + 1],
                )
        if dbg is not None:
            nc.sync.dma_start(out=dbg[0], in_=Hcw.bitcast(FP))
            nc.sync.dma_start(out=dbg[1], in_=Haug.bitcast(FP))


def _stack_uwt(uw_pad, ml_dtypes):
    s = np.zeros((128, LPAD), np.float32)
    s[:C] = uw_pad.T
    s[64 : 64 + C] = uw_pad.T
    return np.ascontiguousarray(s).astype(ml_dtypes.bfloat16)


def host_prep(inputs):
    """Full inputs -> (shared input map, list of 8 per-core input maps)."""
    x = np.asarray(inputs["x"]).astype(np.int32)
    wemb = np.ascontiguousarray(np.asarray(inputs["W_embed"], dtype=np.float32))
    conv_w = np.asarray(inputs["conv_w"], dtype=np.float32)
    conv_b = np.asarray(inputs["conv_b"], dtype=np.float32)
    u_w = np.asarray(inputs["u_w"], dtype=np.float32)
    out_w = np.asarray(inputs["out_w"], dtype=np.float32)
    out_b = np.asarray(inputs["out_b"], dtype=np.float32)

    xp = np.zeros((B, WPAD), np.int32)
    xp[:, :W] = x
    idx = np.ascontiguousarray(
        xp.reshape(B, NCI, 128).transpose(2, 0, 1).reshape(128, B * NCI)
    )
    import ml_dtypes

    convwt = np.ascontiguousarray(
        np.concatenate([conv_w[:, :, k].T for k in range(K)], axis=1)
    ).astype(ml_dtypes.bfloat16)  # (E, K*C)
    convbp = np.ascontiguousarray(conv_b.reshape(C, 1))

    shared = {"x_idx": idx, "wemb": wemb, "convwt": convwt, "convb": convbp}
    if EMB_HOST:
        embf = wemb[xp]  # (B, WPAD, E); pad tokens use row x=0, zeroed on chip
        shared["embx"] = np.ascontiguousarray(
            embf.reshape(B, NCI, 128, E).transpose(0, 2, 1, 3).reshape(B, 128, NCI * E)
        )
    in_maps = []
    for c in range(NCORES):
        l0 = c * LSH
        uw_pad = np.zeros((LPAD, C), np.float32)
        uw_pad[:LSH] = u_w[l0 : l0 + LSH]
        ow_pad = np.zeros((LPAD, C), np.float32)
        ow_pad[:LSH] = out_w[l0 : l0 + LSH]
        ob_pad = np.zeros(LPAD, np.float32)
        ob_pad[:LSH] = out_b[l0 : l0 + LSH]
        import ml_dtypes

        in_maps.append(
            dict(
                shared,
                uwt=_stack_uwt(uw_pad, ml_dtypes),
                owp=np.ascontiguousarray(
                    ow_pad.reshape(LT, 128, C).transpose(1, 0, 2).reshape(128, LT * C)
                ),
                obp=np.ascontiguousarray(ob_pad.reshape(LT, 128).T),
            )
        )
    return in_maps


def unshard(outs):
    """outs: list of 8 arrays (B, 128, LT) -> (B, L)."""
    parts = [
        np.asarray(o).transpose(0, 2, 1).reshape(B, LPAD)[:, :LSH] for o in outs
    ]
    return np.ascontiguousarray(np.concatenate(parts, axis=1), dtype=np.float32)


_NC = None
LAST_RESULTS = None


def kernel(**inputs) -> np.ndarray:
    global _NC, LAST_RESULTS
    in_maps = host_prep(inputs)
    if _NC is None:
        _NC = build_nc(num_devices=NCORES)
    import os

    trace = bool(int(os.environ.get("KERNEL_TRACE", "0")))
    res = run_bass_kernel_spmd(
        _NC, in_maps, core_ids=list(range(NCORES)), trace=trace
    )
    LAST_RESULTS = res
    outs = [res.results[i]["out"] for i in range(NCORES)]
    return unshard(outs)

